# revision 30
# baseline (speedup 1.0000x reference)
"""GCN encoder (VGAE-style, 6 GCNConv) on 8 trn2 NeuronCores — v2.

Strategy: partition nodes across the 8 cores; weights replicated. Tables are
bf16 and chunked 2-way by local-row range so that each layer's AllGather is
issued as 4 sub-AllGathers interleaved with the aggregation pass (the wire
time hides behind the next pass's gathers). Edges are relabeled to per-chunk
table indices (int16-safe), sorted by (dst-group-batch, src-chunk, dst-group,
src) and gathered with one big pad-free dma_gather per (batch, chunk) —
~52 calls/pass instead of ~245 — then segment-summed into PSUM with one-hot
indicator matmuls in bf16 (FWL-enabled). norm factorization: dis[src] is
pre-applied to table rows, dis[dst] at PSUM evacuation.
"""
import numpy as np

import concourse.bass as bass
import concourse.mybir as mybir
import concourse.tile as tile
import concourse.bacc as bacc
from concourse.bass_utils import run_bass_kernel_spmd

P = 128
NCORES = 8
N = 50000
E = 1600000
D = 128           # IN_C == HID == 128
OUTC = 64
NCHUNK = 2        # table sub-AllGather chunks
NQUEUES = 4       # SWDGE queues for gathers (ucode max 4)
GB = 4            # dst groups per gather batch
SUB = 16          # tiles per sub-gather (finer split -> better queue overlap)

SELU_L = 1.0507009873554805
SELU_A = 1.6732632423543772

f32 = mybir.dt.float32
bf16 = mybir.dt.bfloat16
i16 = mybir.dt.int16

_CACHE = {}


def _dims():
    NPC = N // NCORES
    G = (NPC + P - 1) // P
    last_rows = NPC - (G - 1) * P
    base_g = G // NCHUNK
    rem = G - base_g * NCHUNK
    gcounts = [base_g + (1 if c < rem else 0) for c in range(NCHUNK)]
    gstart = np.concatenate([[0], np.cumsum(gcounts)]).astype(np.int64)
    crow_start = np.array([min(int(gstart[c]) * P, NPC)
                           for c in range(NCHUNK + 1)], np.int64)
    crows = [int(crow_start[c + 1] - crow_start[c]) for c in range(NCHUNK)]
    tbase = np.concatenate([[0], np.cumsum([NCORES * r for r in crows])])
    NB = (G + GB - 1) // GB
    return NPC, G, last_rows, gstart, crow_start, crows, tbase, NB


def _set_dims(n, e):
    """Test hook: shrink the problem (n must be divisible by NCORES)."""
    global N, E
    N, E = n, e


def _pack_idx16(vals, ntiles):
    """flat order i -> [128, ntiles*8] int16; i at (i%16, i//16), the 16-row
    block replicated to all 8 gpsimd cores."""
    n = ntiles * P
    blk = np.zeros((16, n // 16), np.int16)
    if len(vals):
        i = np.arange(len(vals))
        blk[i % 16, i // 16] = vals.astype(np.int16)
    return np.tile(blk, (8, 1))


def _preprocess(edge_index):
    NPC, G, last_rows, gstart, crow_start, crows, tbase, NB = _dims()
    src = np.asarray(edge_index[0], dtype=np.int64)
    dst = np.asarray(edge_index[1], dtype=np.int64)
    loops = np.arange(N, dtype=np.int64)
    src = np.concatenate([src, loops])
    dst = np.concatenate([dst, loops])

    deg = np.bincount(dst, minlength=N).astype(np.float32)  # >=1 (self loop)
    dis = 1.0 / np.sqrt(deg)

    # relabel src -> (chunk, relid within chunk table)
    k_s = src // NPC
    r_s = src % NPC
    c_s = np.searchsorted(crow_start, r_s, side="right") - 1
    crows_arr = np.asarray(crows, np.int64)
    relid = k_s * crows_arr[c_s] + (r_s - crow_start[c_s])

    k_d = dst // NPC
    ld = dst % NPC
    g_d = ld // P
    b_d = g_d // GB

    per_core = []
    for k in range(NCORES):
        sel = np.nonzero(k_d == k)[0]
        bb, cc, gg = b_d[sel], c_s[sel], g_d[sel]
        rel, ldd = relid[sel], ld[sel]
        order = np.lexsort((rel, gg, cc, bb))
        bb, cc, gg, rel, ldd = (a[order] for a in (bb, cc, gg, rel, ldd))

        cell = (bb * NCHUNK + cc)
        counts = np.bincount(cell, minlength=NB * NCHUNK)
        cell_start = np.concatenate([[0], np.cumsum(counts)])

        T = np.zeros((NB, NCHUNK), np.int64)      # tiles per (batch, chunk)
        idx_parts = []
        jobs = [[] for _ in range(NB)]            # per batch: (c, t, g, col)
        dst_cols = []                             # per job: 128 local-dst vals
        dis_cols = []                             # per job: dis[dst] per row
        for b in range(NB):
            for c in range(NCHUNK):
                cidx = b * NCHUNK + c
                s0, cnt = cell_start[cidx], counts[cidx]
                t_bc = -(-cnt // P) if cnt else 0
                T[b, c] = t_bc
                if not t_bc:
                    continue
                rv = np.full(t_bc * P, -1, np.int64)
                rv[:cnt] = rel[s0:s0 + cnt]
                idx_parts.append(_pack_idx16(rv, t_bc))
                gv = gg[s0:s0 + cnt]
                ld_cell = ldd[s0:s0 + cnt]
                lv = ld_cell - gv * P             # local dst within group
                # group spans -> jobs
                ug, ginds = np.unique(gv, return_index=True)
                gends = np.concatenate([ginds[1:], [cnt]])
                for g, a0, a1 in zip(ug, ginds, gends):
                    t0, t1 = a0 // P, (a1 - 1) // P
                    for t in range(t0, t1 + 1):
                        e0, e1 = max(a0, t * P), min(a1, (t + 1) * P)
                        col = np.full(P, 500.0, np.float32)
                        col[e0 - t * P:e1 - t * P] = lv[e0:e1]
                        dcol = np.ones(P, np.float32)
                        dcol[e0 - t * P:e1 - t * P] = dis[
                            k * NPC + ld_cell[e0:e1]]
                        jobs[b].append((c, t, int(g), len(dst_cols)))
                        dst_cols.append(col)
                        dis_cols.append(dcol)
        idx = (np.concatenate(idx_parts, axis=1) if idx_parts
               else np.zeros((P, 8), np.int16))
        dstloc = np.stack(dst_cols, axis=1) if dst_cols else np.zeros((P, 1))
        dstdis = np.stack(dis_cols, axis=1) if dis_cols else np.ones((P, 1))
        dis_k = np.zeros((P, G), np.float32)
        dis_k.T.flat[:NPC] = dis[k * NPC:(k + 1) * NPC]
        per_core.append(dict(T=T, jobs=jobs, idx=idx,
                             cnt=counts.reshape(NB, NCHUNK),
                             dstloc=dstloc.astype(np.float32),
                             dstdis=dstdis.astype(np.float32), dis=dis_k))

    # all cores must share one device program: use per-(b,c) max tiles and
    # the union job structure? -> instead pad each core's plan to a common
    # structural "shape": T_max per (b,c) and per-batch job list built from
    # T_max tiles. Jobs differ per core in (g, col) mapping, which is data
    # (dstloc) not structure, as long as job COUNT and (c, t, psum-slot)
    # sequences match. We force a common structure by padding:
    Tmax = np.maximum.reduce([pc["T"] for pc in per_core])
    plan = _common_plan(Tmax, per_core, NB)
    plan["cnt_min"] = np.minimum.reduce([pc["cnt"] for pc in per_core])
    return plan, per_core, dis


def _common_plan(Tmax, per_core, NB):
    """Build one structural plan all cores share.

    Per (b,c): Tmax[b,c] tiles. Per batch, the job sequence is the union
    shape: for each (c,t) in order, the set of psum slots any core touches.
    Each core then maps its own (g,col) data into this shape; cores lacking
    a (c,t,slot) job use an all-500 dstloc column (zero indicator).
    """
    NCHUNKl = Tmax.shape[1]
    plan_jobs = []                          # per batch: list of (c, t, slot)
    for b in range(NB):
        slots_at = {}
        for pc in per_core:
            for (c, t, g, col) in pc["jobs"][b]:
                slots_at.setdefault((c, t), set()).add(g - b * GB)
        # chunk-major: all chunk-0 jobs first so the PE only needs chunk 1's
        # (later-arriving) table halfway through the batch
        all_slots = sorted({s for v in slots_at.values() for s in v} or {0})
        seq = []
        for c in range(NCHUNKl):
            for s in all_slots:
                for t in range(int(Tmax[b, c])):
                    if s in slots_at.get((c, t), ()) or not slots_at:
                        seq.append((c, t, s))
        plan_jobs.append(seq)
    return dict(Tmax=Tmax, jobs=plan_jobs, NB=NB, nchunk=NCHUNK, gb=GB)


def _core_inputs(plan, pc):
    """Map one core's (idx, dstloc) into the common plan shape."""
    Tmax, NB = plan["Tmax"], plan["NB"]
    NCH = Tmax.shape[1]
    T = pc["T"]
    # idx: concat per (b,c) padded to Tmax[b,c] tiles
    parts = []
    off = 0
    for b in range(NB):
        for c in range(NCH):
            t_bc = int(T[b, c])
            tm = int(Tmax[b, c])
            if t_bc:
                parts.append(pc["idx"][:, off * 8:(off + t_bc) * 8])
                off += t_bc
            if tm > t_bc:
                parts.append(np.full((P, (tm - t_bc) * 8), -1, np.int16))
    idx = np.concatenate(parts, axis=1) if parts else np.zeros((P, 8), np.int16)

    # dstloc/dstdis: one column per plan job
    cols = []
    dcols = []
    for b in range(NB):
        mine = {(c, t, g - b * GB): col for (c, t, g, col) in pc["jobs"][b]}
        for (c, t, s) in plan["jobs"][b]:
            ci = mine.get((c, t, s))
            cols.append(pc["dstloc"][:, ci] if ci is not None
                        else np.full(P, 500.0, np.float32))
            dcols.append(pc["dstdis"][:, ci] if ci is not None
                         else np.ones(P, np.float32))
    dstloc = np.stack(cols, axis=1).astype(np.float32)
    dstdis = np.stack(dcols, axis=1).astype(np.float32)
    # per-sub-gather valid counts, host-flattened in (b, c, s) order. A sub
    # entirely past this core's count gets nv=1 with its first index patched
    # from -1 to 0: one harmless row-0 fetch into the zero-indicator padding
    # (nv=1 with idx=-1 is a malformed descriptor and hangs the DMA).
    vals = []
    off = 0
    for b in range(NB):
        for c in range(NCH):
            tm = int(Tmax[b, c])
            cnt = int(pc["cnt"][b, c])
            for s in range((tm + SUB - 1) // SUB if tm else 0):
                rows_sub = min(SUB, tm - s * SUB) * P
                v = min(cnt - s * SUB * P, rows_sub)
                if v < 1:
                    v = 1
                    idx[0::16, (off + s * SUB) * 8] = 0
                vals.append(v)
            off += tm
    gcnt = np.asarray(vals or [1], np.int32).reshape(1, -1)
    return idx, dstloc, dstdis, gcnt


# ------------------------------------------------------------ device program
def _build(plan, use_bias, sim_safe=False, repeat=1, skip_ag=False,
           n_passes=5, skip_gather=False, skip_ind=False, skip_mm=False):
    NPC, G, last_rows, gstart, crow_start, crows, tbase, NB = _dims()
    Tmax, jobs_all = plan["Tmax"], plan["jobs"]
    TOT_TILES = int(Tmax.sum())
    NJOBS = sum(len(j) for j in jobs_all)
    TMAXBUF = int(Tmax.max())

    nc = bacc.Bacc("TRN2", target_bir_lowering=False, debug=False,
                   enable_asserts=False, num_devices=NCORES,
                   num_swdge_queues=NQUEUES)

    NSUBTOT = sum((int(Tmax[b, c]) + SUB - 1) // SUB
                  for b in range(NB) for c in range(NCHUNK) if Tmax[b, c])

    def inp(name, shape, dt=f32):
        return nc.dram_tensor(name, shape, dt, kind="ExternalInput")

    idx_in = inp("idx", [P, TOT_TILES * 8], i16)
    gcnt_in = inp("gcnt", [1, max(NSUBTOT, 1)], mybir.dt.int32)
    dstloc_in = inp("dstloc", [P, NJOBS])
    dstdis_in = inp("dstdis", [P, NJOBS])
    iota_in = inp("iota", [P, P], bf16)
    dis_in = inp("dis_sc", [P, G])
    xT_in = inp("xT", [P, G * P], bf16)
    w_in = [inp(f"W{i}", [P, P], bf16) for i in range(5)]  # W0..W3, Wmulv
    bb_in = ([inp(f"BB{i}", [P, 1]) for i in range(4)]
             + [inp("BB4", [P, P])]) if use_bias else []

    mu_out = nc.dram_tensor("mu_out", [NPC, OUTC], f32, kind="ExternalOutput")
    lv_out = nc.dram_tensor("lv_out", [NPC, OUTC], f32, kind="ExternalOutput")

    h_own = [nc.dram_tensor(f"h_own{c}", [crows[c], D], bf16)
             for c in range(NCHUNK)]
    tbl = [[nc.dram_tensor(f"table{p}_{c}", [NCORES * crows[c], D], bf16,
                           addr_space="Shared")
            for c in range(NCHUNK)] for p in range(2)]

    RG = [list(range(NCORES))]
    AF = mybir.ActivationFunctionType

    # chunk of group g (for own-rows routing)
    def chunk_of_group(g):
        for c in range(NCHUNK):
            if gstart[c] <= g < gstart[c + 1]:
                return c
        raise AssertionError

    # tile offset of (b, c) in idx / msg space
    tile_off = np.zeros((NB, NCHUNK), np.int64)
    acc = 0
    for b in range(NB):
        for c in range(NCHUNK):
            tile_off[b, c] = acc
            acc += int(Tmax[b, c])

    with tile.TileContext(nc) as tc:
        with (
            tc.tile_pool(name="const", bufs=1) as cpool,
            tc.tile_pool(name="msg", bufs=3 * NCHUNK) as msg_pool,
            tc.tile_pool(name="ind", bufs=6) as ind_pool,
            tc.tile_pool(name="act", bufs=3) as act_pool,
            tc.tile_pool(name="tmp", bufs=4) as tmp_pool,
            tc.tile_pool(name="hps", bufs=3, space="PSUM") as agg_psum,
            tc.tile_pool(name="zps", bufs=2, space="PSUM") as z_psum,
        ):
            def load(ap_in, shape, tag, dt=f32):
                t = cpool.tile(shape, dt, tag=tag)
                nc.sync.dma_start(out=t[:], in_=ap_in[:, :])
                return t

            idx = load(idx_in, [P, TOT_TILES * 8], "c_idx", i16)
            gcnt = load(gcnt_in, [1, max(NSUBTOT, 1)], "c_gcnt",
                        mybir.dt.int32)
            dstloc = load(dstloc_in, [P, NJOBS], "c_dstloc")
            dstdis = load(dstdis_in, [P, NJOBS], "c_dstdis")
            cnt_regs = [nc.gpsimd.alloc_register(f"cntreg{i}")
                        for i in range(4)]
            cnt_rr = [0]

            def load_cnt(col):
                r = cnt_regs[cnt_rr[0] % 4]
                cnt_rr[0] += 1
                nc.gpsimd.reg_load(r, gcnt[0:1, col:col + 1])
                return r
            iota = load(iota_in, [P, P], "c_iota", bf16)
            dis_sc = load(dis_in, [P, G], "c_dis")
            xT = load(xT_in, [P, G * P], "c_xT", bf16)
            W = [load(w, [P, P], f"c_W{i}", bf16) for i, w in enumerate(w_in)]
            BB = ([load(b, [P, 1], f"c_BB{i}") for i, b in enumerate(bb_in[:4])]
                  + [load(bb_in[4], [P, P], "c_BB4")] if use_bias else None)

            def own_rows(g, z_ps, pi):
                """scale z (PSUM [128, D]) by dis, store bf16 rows, fire AGs."""
                rows = P if g < G - 1 else last_rows
                c = chunk_of_group(g)
                r0 = g * P - int(crow_start[c])
                h = act_pool.tile([P, D], bf16, tag="hrow")
                nc.scalar.mul(h[:], z_ps[:], dis_sc[:, g:g + 1])
                nc.sync.dma_start(out=h_own[c][r0:r0 + rows, :],
                                  in_=h[:rows, :])
                if g == int(gstart[c + 1]) - 1 and not skip_ag:
                    nc.gpsimd.collective_compute(
                        "AllGather", mybir.AluOpType.bypass,
                        replica_groups=RG,
                        ins=[h_own[c].ap().opt()],
                        outs=[tbl[pi % 2][c].ap().opt()])

            for _rep in range(repeat):
                # ---- prologue: table0 rows = dis * (x @ W0)
                for g in range(G):
                    z = z_psum.tile([P, D], f32, space="PSUM")
                    nc.tensor.matmul(out=z[:], lhsT=xT[:, g * P:(g + 1) * P],
                                     rhs=W[0][:], start=True, stop=True)
                    own_rows(g, z, 0)

                passes = [("selu", 1), ("silu", 2), ("silu", 3),
                          ("softplus_neg", 4), ("final", None)]
                passes = passes[:n_passes]
                for pi, (fn, wnext) in enumerate(passes):
                    tblp = tbl[pi % 2]
                    subcol = [0]
                    for b in range(NB):
                        glo = b * GB
                        ghi = min(glo + GB, G)
                        ng = ghi - glo
                        # ---- gathers: one per (batch, chunk)
                        msgs = []
                        for c in range(NCHUNK):
                            tm = int(Tmax[b, c])
                            if not tm:
                                msgs.append(None)
                                continue
                            m = msg_pool.tile([P, TMAXBUF, D], bf16)
                            o = int(tile_off[b, c])
                            t0m = int(plan["cnt_min"][b, c]) // P
                            if t0m < tm:
                                nc.vector.memset(m[:, t0m:tm, :], 0.0)
                            for s in range((tm + SUB - 1) // SUB):
                                t0s = s * SUB
                                t1s = min(t0s + SUB, tm)
                                nt = t1s - t0s
                                if not skip_gather:
                                    nv = load_cnt(subcol[0])
                                    # chunk c owns its own queue subset: a
                                    # pending AG on chunk 1 never blocks
                                    # chunk-0 gathers
                                    if NQUEUES >= 2 * NCHUNK:
                                        qpc = NQUEUES // NCHUNK
                                        qn = c * qpc + s % qpc
                                    else:
                                        qn = subcol[0] % NQUEUES
                                    nc.gpsimd.dma_gather(
                                        m[:, t0s:t1s, :],
                                        tblp[c][0:NCORES * crows[c], :],
                                        idx[:, (o + t0s) * 8:(o + t1s) * 8],
                                        nt * P, nv, D,
                                        single_packet=nt * P <= 128,
                                        queue_num=qn)
                                subcol[0] += 1
                            msgs.append(m)
                        # ---- indicator matmuls
                        psb = agg_psum.tile([P, GB * D], f32, space="PSUM",
                                            name="aggps", tag="aggps")
                        ps = [psb[:, s * D:(s + 1) * D] for s in range(ng)]
                        seq = jobs_all[b]
                        first = {}
                        last = {}
                        for j, (c, t, s) in enumerate(seq):
                            first.setdefault(s, j)
                            last[s] = j
                        col0 = sum(len(jobs_all[x]) for x in range(b))
                        flipped = wnext is not None
                        for j, (c, t, s) in enumerate(seq):
                            if s >= ng:
                                continue
                            ind = ind_pool.tile([P, P], bf16)
                            if not skip_ind:
                                # indicator with dis[dst] folded in:
                                # ind[e, d] = (d == dst[e]) * dis[dst[e]]
                                nc.vector.tensor_scalar(
                                    out=ind[:], in0=iota[:],
                                    scalar1=dstloc[:, col0 + j:col0 + j + 1],
                                    scalar2=dstdis[:, col0 + j:col0 + j + 1],
                                    op0=mybir.AluOpType.is_equal,
                                    op1=mybir.AluOpType.mult)
                            if not skip_mm or first[s] == j or last[s] == j:
                                if flipped:
                                    # psT[f, d] += msg[e, f]^T @ ind[e, d]
                                    nc.tensor.matmul(out=ps[s][:],
                                                     lhsT=msgs[c][:, t, :],
                                                     rhs=ind[:],
                                                     start=(first[s] == j),
                                                     stop=(last[s] == j))
                                else:
                                    # ps[d, f] += ind[e, d]^T @ msg[e, f]
                                    nc.tensor.matmul(out=ps[s][:], lhsT=ind[:],
                                                     rhs=msgs[c][:, t, :],
                                                     start=(first[s] == j),
                                                     stop=(last[s] == j))
                        # ---- evacuation per group (dis already folded into
                        # the indicator). Middle passes: psT [f, d]; act then
                        # serves directly as lhsT for the next W matmul.
                        for s in range(ng):
                            g = glo + s
                            pg = ps[s]
                            act = act_pool.tile([P, D],
                                                bf16 if flipped else f32,
                                                tag="act")
                            if use_bias and flipped:
                                lin = tmp_pool.tile([P, D], f32, tag="lin")
                                nc.vector.tensor_scalar(
                                    out=lin[:], in0=pg[:],
                                    scalar1=BB[pi][:, 0:1], scalar2=None,
                                    op0=mybir.AluOpType.add)
                                srcx = lin
                            elif use_bias:
                                lin = tmp_pool.tile([P, D], f32, tag="lin")
                                nc.vector.tensor_tensor(
                                    out=lin[:], in0=pg[:], in1=BB[pi][:],
                                    op=mybir.AluOpType.add)
                                srcx = lin
                            else:
                                srcx = pg
                            if fn == "silu":
                                if sim_safe:
                                    sg = tmp_pool.tile([P, D], f32, tag="sg")
                                    xx = tmp_pool.tile([P, D], f32, tag="xx")
                                    nc.scalar.activation(sg[:], srcx[:],
                                                         AF.Sigmoid)
                                    nc.scalar.mul(xx[:], srcx[:], 1.0)
                                    nc.vector.tensor_tensor(
                                        out=act[:], in0=sg[:], in1=xx[:],
                                        op=mybir.AluOpType.mult)
                                else:
                                    nc.scalar.activation(act[:], srcx[:],
                                                         AF.Silu)
                            elif fn == "softplus_neg":
                                e = tmp_pool.tile([P, D], f32, tag="sp_e")
                                nc.scalar.activation(e[:], srcx[:], AF.Exp,
                                                     scale=-1.0)
                                nc.scalar.activation(act[:], e[:], AF.Ln,
                                                     bias=1.0)
                            elif fn == "selu":
                                r = tmp_pool.tile([P, D], f32, tag="selu_r")
                                m = tmp_pool.tile([P, D], f32, tag="selu_m")
                                nc.scalar.activation(r[:], srcx[:], AF.Relu,
                                                     scale=SELU_L)
                                nc.scalar.activation(m[:], srcx[:], AF.Relu,
                                                     scale=-1.0)
                                nc.scalar.activation(m[:], m[:], AF.Exp,
                                                     scale=-1.0)
                                nc.vector.tensor_scalar(
                                    out=m[:], in0=m[:],
                                    scalar1=SELU_L * SELU_A,
                                    scalar2=-SELU_L * SELU_A,
                                    op0=mybir.AluOpType.mult,
                                    op1=mybir.AluOpType.add)
                                nc.vector.tensor_tensor(
                                    out=act[:], in0=r[:], in1=m[:],
                                    op=mybir.AluOpType.add)
                            else:  # final (unflipped)
                                nc.vector.tensor_copy(act[:], srcx[:])

                            rows = P if g < G - 1 else last_rows
                            if wnext is None:
                                nc.sync.dma_start(
                                    out=mu_out[g * P:g * P + rows, :],
                                    in_=act[:rows, 0:OUTC])
                                nc.sync.dma_start(
                                    out=lv_out[g * P:g * P + rows, :],
                                    in_=act[:rows, OUTC:D])
                            else:
                                # act [f, d] bf16 is directly the next lhsT
                                z = z_psum.tile([P, D], f32, space="PSUM")
                                nc.tensor.matmul(out=z[:], lhsT=act[:],
                                                 rhs=W[wnext][:],
                                                 start=True, stop=True)
                                own_rows(g, z, pi + 1)
    nc.finalize()
    return nc


# ------------------------------------------------------------------- driver
def _make_in_maps(x, plan, per_core, Ws, biases=None):
    NPC, G, last_rows, *_ = _dims()
    iota = np.tile(np.arange(P, dtype=np.float32), (P, 1))
    bfdt = mybir.dt.np(bf16)
    in_maps = []
    for k in range(NCORES):
        pc = per_core[k]
        idx, dstloc, dstdis, gcnt = _core_inputs(plan, pc)
        dis_k = pc["dis"]
        xT = np.zeros((P, G * P), np.float32)
        xT[:, :NPC] = x[k * NPC:(k + 1) * NPC].T
        m = dict(idx=idx, gcnt=gcnt, dstloc=dstloc.astype(np.float32),
                 dstdis=dstdis.astype(np.float32),
                 iota=iota.astype(bfdt), dis_sc=dis_k,
                 xT=xT.astype(bfdt))
        for i, w in enumerate(Ws):
            m[f"W{i}"] = np.asarray(w, np.float32).astype(bfdt)
        if biases is not None:
            for i, b in enumerate(biases):
                bv = np.asarray(b, dtype=np.float32)
                if i < 4:
                    m[f"BB{i}"] = bv.reshape(P, 1)
                else:
                    m[f"BB{i}"] = np.tile(bv[None, :], (P, 1))
        in_maps.append(m)
    return in_maps


def kernel(x, edge_index, W0, b0, W1, b1, W2, b2, W3, b3, Wmu, bmu, Wlv, blv):
    x = np.asarray(x, dtype=np.float32)
    edge_index = np.asarray(edge_index)
    assert x.shape == (N, D) and edge_index.shape == (2, E)

    plan, per_core, _dis = _preprocess(edge_index)
    use_bias = any(np.any(np.asarray(b)) for b in (b0, b1, b2, b3, bmu, blv))

    key = (plan["Tmax"].tobytes(),
           tuple(tuple(j) for j in plan["jobs"][0]), use_bias)
    if key not in _CACHE:
        _CACHE[key] = _build(plan, use_bias)
    nc = _CACHE[key]

    Wmulv = np.concatenate([-np.asarray(Wmu), -np.asarray(Wlv)],
                           axis=1).astype(np.float32)
    Ws = [np.asarray(w, dtype=np.float32) for w in (W0, W1, W2, W3)] + [Wmulv]
    biases = None
    if use_bias:
        bmulv = np.concatenate([np.asarray(bmu), np.asarray(blv)])
        biases = (b0, b1, b2, b3, bmulv)
    in_maps = _make_in_maps(x, plan, per_core, Ws, biases)

    res = run_bass_kernel_spmd(nc, in_maps, core_ids=list(range(NCORES)))
    mu = np.concatenate([res.results[k]["mu_out"] for k in range(NCORES)],
                        axis=0)
    lv = np.concatenate([res.results[k]["lv_out"] for k in range(NCORES)],
                        axis=0)
    return (mu, lv)



# revision 40
# speedup vs baseline: 1.0838x; 1.0838x over previous
"""GCN encoder (VGAE-style, 6 GCNConv) on 8 trn2 NeuronCores — v2.

Strategy: partition nodes across the 8 cores; weights replicated. Tables are
bf16 and chunked 2-way by local-row range so that each layer's AllGather is
issued as 4 sub-AllGathers interleaved with the aggregation pass (the wire
time hides behind the next pass's gathers). Edges are relabeled to per-chunk
table indices (int16-safe), sorted by (dst-group-batch, src-chunk, dst-group,
src) and gathered with one big pad-free dma_gather per (batch, chunk) —
~52 calls/pass instead of ~245 — then segment-summed into PSUM with one-hot
indicator matmuls in bf16 (FWL-enabled). norm factorization: dis[src] is
pre-applied to table rows, dis[dst] at PSUM evacuation.
"""
import numpy as np

import concourse.bass as bass
import concourse.mybir as mybir
import concourse.tile as tile
import concourse.bacc as bacc
from concourse.bass_utils import run_bass_kernel_spmd

P = 128
NCORES = 8
N = 50000
E = 1600000
D = 128           # IN_C == HID == 128
OUTC = 64
NCHUNK = 2        # table sub-AllGather chunks
NQUEUES = 4       # SWDGE queues for gathers (ucode max 4)
GB = 4            # dst groups per gather batch
SUB = 16          # tiles per sub-gather (finer split -> better queue overlap)
SEQ_CMAJOR = True   # per-batch job order: chunk-major (vs slot-major)
QSPLIT = False      # dedicate queue subsets per chunk
PSUM_BUFS = 3       # agg psum buffers

SELU_L = 1.0507009873554805
SELU_A = 1.6732632423543772

f32 = mybir.dt.float32
bf16 = mybir.dt.bfloat16
i16 = mybir.dt.int16

_CACHE = {}


def _dims():
    NPC = N // NCORES
    G = (NPC + P - 1) // P
    last_rows = NPC - (G - 1) * P
    base_g = G // NCHUNK
    rem = G - base_g * NCHUNK
    gcounts = [base_g + (1 if c < rem else 0) for c in range(NCHUNK)]
    gstart = np.concatenate([[0], np.cumsum(gcounts)]).astype(np.int64)
    crow_start = np.array([min(int(gstart[c]) * P, NPC)
                           for c in range(NCHUNK + 1)], np.int64)
    crows = [int(crow_start[c + 1] - crow_start[c]) for c in range(NCHUNK)]
    tbase = np.concatenate([[0], np.cumsum([NCORES * r for r in crows])])
    NB = (G + GB - 1) // GB
    return NPC, G, last_rows, gstart, crow_start, crows, tbase, NB


def _set_dims(n, e):
    """Test hook: shrink the problem (n must be divisible by NCORES)."""
    global N, E
    N, E = n, e


def _pack_idx16(vals, ntiles):
    """flat order i -> [128, ntiles*8] int16; i at (i%16, i//16), the 16-row
    block replicated to all 8 gpsimd cores."""
    n = ntiles * P
    blk = np.zeros((16, n // 16), np.int16)
    if len(vals):
        i = np.arange(len(vals))
        blk[i % 16, i // 16] = vals.astype(np.int16)
    return np.tile(blk, (8, 1))


def _preprocess(edge_index):
    NPC, G, last_rows, gstart, crow_start, crows, tbase, NB = _dims()
    src = np.asarray(edge_index[0], dtype=np.int64)
    dst = np.asarray(edge_index[1], dtype=np.int64)
    loops = np.arange(N, dtype=np.int64)
    src = np.concatenate([src, loops])
    dst = np.concatenate([dst, loops])

    deg = np.bincount(dst, minlength=N).astype(np.float32)  # >=1 (self loop)
    dis = 1.0 / np.sqrt(deg)

    # relabel src -> (chunk, relid within chunk table)
    k_s = src // NPC
    r_s = src % NPC
    c_s = np.searchsorted(crow_start, r_s, side="right") - 1
    crows_arr = np.asarray(crows, np.int64)
    relid = k_s * crows_arr[c_s] + (r_s - crow_start[c_s])

    k_d = dst // NPC
    ld = dst % NPC
    g_d = ld // P
    b_d = g_d // GB

    per_core = []
    for k in range(NCORES):
        sel = np.nonzero(k_d == k)[0]
        bb, cc, gg = b_d[sel], c_s[sel], g_d[sel]
        rel, ldd = relid[sel], ld[sel]
        order = np.lexsort((rel, gg, cc, bb))
        bb, cc, gg, rel, ldd = (a[order] for a in (bb, cc, gg, rel, ldd))

        cell = (bb * NCHUNK + cc)
        counts = np.bincount(cell, minlength=NB * NCHUNK)
        cell_start = np.concatenate([[0], np.cumsum(counts)])

        T = np.zeros((NB, NCHUNK), np.int64)      # tiles per (batch, chunk)
        idx_parts = []
        jobs = [[] for _ in range(NB)]            # per batch: (c, t, g, col)
        dst_cols = []                             # per job: 128 local-dst vals
        dis_cols = []                             # per job: dis[dst] per row
        for b in range(NB):
            for c in range(NCHUNK):
                cidx = b * NCHUNK + c
                s0, cnt = cell_start[cidx], counts[cidx]
                t_bc = -(-cnt // P) if cnt else 0
                T[b, c] = t_bc
                if not t_bc:
                    continue
                rv = np.full(t_bc * P, -1, np.int64)
                rv[:cnt] = rel[s0:s0 + cnt]
                idx_parts.append(_pack_idx16(rv, t_bc))
                gv = gg[s0:s0 + cnt]
                ld_cell = ldd[s0:s0 + cnt]
                lv = ld_cell - gv * P             # local dst within group
                # group spans -> jobs
                ug, ginds = np.unique(gv, return_index=True)
                gends = np.concatenate([ginds[1:], [cnt]])
                for g, a0, a1 in zip(ug, ginds, gends):
                    t0, t1 = a0 // P, (a1 - 1) // P
                    for t in range(t0, t1 + 1):
                        e0, e1 = max(a0, t * P), min(a1, (t + 1) * P)
                        col = np.full(P, 500.0, np.float32)
                        col[e0 - t * P:e1 - t * P] = lv[e0:e1]
                        dcol = np.ones(P, np.float32)
                        dcol[e0 - t * P:e1 - t * P] = dis[
                            k * NPC + ld_cell[e0:e1]]
                        jobs[b].append((c, t, int(g), len(dst_cols)))
                        dst_cols.append(col)
                        dis_cols.append(dcol)
        idx = (np.concatenate(idx_parts, axis=1) if idx_parts
               else np.zeros((P, 8), np.int16))
        dstloc = np.stack(dst_cols, axis=1) if dst_cols else np.zeros((P, 1))
        dstdis = np.stack(dis_cols, axis=1) if dis_cols else np.ones((P, 1))
        dis_k = np.zeros((P, G), np.float32)
        dis_k.T.flat[:NPC] = dis[k * NPC:(k + 1) * NPC]
        per_core.append(dict(T=T, jobs=jobs, idx=idx,
                             cnt=counts.reshape(NB, NCHUNK),
                             dstloc=dstloc.astype(np.float32),
                             dstdis=dstdis.astype(np.float32), dis=dis_k))

    # all cores must share one device program: use per-(b,c) max tiles and
    # the union job structure? -> instead pad each core's plan to a common
    # structural "shape": T_max per (b,c) and per-batch job list built from
    # T_max tiles. Jobs differ per core in (g, col) mapping, which is data
    # (dstloc) not structure, as long as job COUNT and (c, t, psum-slot)
    # sequences match. We force a common structure by padding:
    Tmax = np.maximum.reduce([pc["T"] for pc in per_core])
    plan = _common_plan(Tmax, per_core, NB)
    plan["cnt_min"] = np.minimum.reduce([pc["cnt"] for pc in per_core])
    return plan, per_core, dis


def _common_plan(Tmax, per_core, NB):
    """Build one structural plan all cores share.

    Per (b,c): Tmax[b,c] tiles. Per batch, the job sequence is the union
    shape: for each (c,t) in order, the set of psum slots any core touches.
    Each core then maps its own (g,col) data into this shape; cores lacking
    a (c,t,slot) job use an all-500 dstloc column (zero indicator).
    """
    NCHUNKl = Tmax.shape[1]
    plan_jobs = []                          # per batch: list of (c, t, slot)
    for b in range(NB):
        slots_at = {}
        for pc in per_core:
            for (c, t, g, col) in pc["jobs"][b]:
                slots_at.setdefault((c, t), set()).add(g - b * GB)
        # chunk-major: all chunk-0 jobs first so the PE only needs chunk 1's
        # (later-arriving) table halfway through the batch
        all_slots = sorted({s for v in slots_at.values() for s in v} or {0})
        seq = []
        if SEQ_CMAJOR:
            for c in range(NCHUNKl):
                for s in all_slots:
                    for t in range(int(Tmax[b, c])):
                        if s in slots_at.get((c, t), ()) or not slots_at:
                            seq.append((c, t, s))
        else:
            for s in all_slots:
                for c in range(NCHUNKl):
                    for t in range(int(Tmax[b, c])):
                        if s in slots_at.get((c, t), ()) or not slots_at:
                            seq.append((c, t, s))
        plan_jobs.append(seq)
    return dict(Tmax=Tmax, jobs=plan_jobs, NB=NB, nchunk=NCHUNK, gb=GB)


def _core_inputs(plan, pc):
    """Map one core's (idx, dstloc) into the common plan shape."""
    Tmax, NB = plan["Tmax"], plan["NB"]
    NCH = Tmax.shape[1]
    T = pc["T"]
    # idx: concat per (b,c) padded to Tmax[b,c] tiles
    parts = []
    off = 0
    for b in range(NB):
        for c in range(NCH):
            t_bc = int(T[b, c])
            tm = int(Tmax[b, c])
            if t_bc:
                parts.append(pc["idx"][:, off * 8:(off + t_bc) * 8])
                off += t_bc
            if tm > t_bc:
                parts.append(np.full((P, (tm - t_bc) * 8), -1, np.int16))
    idx = np.concatenate(parts, axis=1) if parts else np.zeros((P, 8), np.int16)

    # dstloc/dstdis: one column per plan job
    cols = []
    dcols = []
    for b in range(NB):
        mine = {(c, t, g - b * GB): col for (c, t, g, col) in pc["jobs"][b]}
        for (c, t, s) in plan["jobs"][b]:
            ci = mine.get((c, t, s))
            cols.append(pc["dstloc"][:, ci] if ci is not None
                        else np.full(P, 500.0, np.float32))
            dcols.append(pc["dstdis"][:, ci] if ci is not None
                         else np.ones(P, np.float32))
    dstloc = np.stack(cols, axis=1).astype(np.float32)
    dstdis = np.stack(dcols, axis=1).astype(np.float32)
    # per-sub-gather valid counts, host-flattened in (b, c, s) order. A sub
    # entirely past this core's count gets nv=1 with its first index patched
    # from -1 to 0: one harmless row-0 fetch into the zero-indicator padding
    # (nv=1 with idx=-1 is a malformed descriptor and hangs the DMA).
    vals = []
    off = 0
    for b in range(NB):
        for c in range(NCH):
            tm = int(Tmax[b, c])
            cnt = int(pc["cnt"][b, c])
            for s in range((tm + SUB - 1) // SUB if tm else 0):
                rows_sub = min(SUB, tm - s * SUB) * P
                v = min(cnt - s * SUB * P, rows_sub)
                if v < 1:
                    v = 1
                    idx[0::16, (off + s * SUB) * 8] = 0
                vals.append(v)
            off += tm
    gcnt = np.asarray(vals or [1], np.int32).reshape(1, -1)
    return idx, dstloc, dstdis, gcnt


# ------------------------------------------------------------ device program
def _build(plan, use_bias, sim_safe=False, repeat=1, skip_ag=False,
           n_passes=5, skip_gather=False, skip_ind=False, skip_mm=False):
    NPC, G, last_rows, gstart, crow_start, crows, tbase, NB = _dims()
    Tmax, jobs_all = plan["Tmax"], plan["jobs"]
    TOT_TILES = int(Tmax.sum())
    NJOBS = sum(len(j) for j in jobs_all)
    TMAXBUF = int(Tmax.max())

    nc = bacc.Bacc("TRN2", target_bir_lowering=False, debug=False,
                   enable_asserts=False, num_devices=NCORES,
                   num_swdge_queues=NQUEUES)

    NSUBTOT = sum((int(Tmax[b, c]) + SUB - 1) // SUB
                  for b in range(NB) for c in range(NCHUNK) if Tmax[b, c])

    def inp(name, shape, dt=f32):
        return nc.dram_tensor(name, shape, dt, kind="ExternalInput")

    idx_in = inp("idx", [P, TOT_TILES * 8], i16)
    gcnt_in = inp("gcnt", [1, max(NSUBTOT, 1)], mybir.dt.int32)
    dstloc_in = inp("dstloc", [P, NJOBS])
    dstdis_in = inp("dstdis", [P, NJOBS])
    iota_in = inp("iota", [P, P], bf16)
    dis_in = inp("dis_sc", [P, G])
    xT_in = inp("xT", [P, G * P], bf16)
    w_in = [inp(f"W{i}", [P, P], bf16) for i in range(5)]  # W0..W3, Wmulv
    bb_in = ([inp(f"BB{i}", [P, 1]) for i in range(4)]
             + [inp("BB4", [P, P])]) if use_bias else []

    mu_out = nc.dram_tensor("mu_out", [NPC, OUTC], f32, kind="ExternalOutput")
    lv_out = nc.dram_tensor("lv_out", [NPC, OUTC], f32, kind="ExternalOutput")

    h_own = [nc.dram_tensor(f"h_own{c}", [crows[c], D], bf16)
             for c in range(NCHUNK)]
    tbl = [[nc.dram_tensor(f"table{p}_{c}", [NCORES * crows[c], D], bf16,
                           addr_space="Shared")
            for c in range(NCHUNK)] for p in range(2)]

    RG = [list(range(NCORES))]
    AF = mybir.ActivationFunctionType

    # chunk of group g (for own-rows routing)
    def chunk_of_group(g):
        for c in range(NCHUNK):
            if gstart[c] <= g < gstart[c + 1]:
                return c
        raise AssertionError

    # tile offset of (b, c) in idx / msg space
    tile_off = np.zeros((NB, NCHUNK), np.int64)
    acc = 0
    for b in range(NB):
        for c in range(NCHUNK):
            tile_off[b, c] = acc
            acc += int(Tmax[b, c])

    with tile.TileContext(nc) as tc:
        with (
            tc.tile_pool(name="const", bufs=1) as cpool,
            tc.tile_pool(name="msg", bufs=3 * NCHUNK) as msg_pool,
            tc.tile_pool(name="ind", bufs=12) as ind_pool,
            tc.tile_pool(name="act", bufs=3) as act_pool,
            tc.tile_pool(name="tmp", bufs=4) as tmp_pool,
            tc.tile_pool(name="hps", bufs=PSUM_BUFS, space="PSUM") as agg_psum,
            tc.tile_pool(name="zps", bufs=2, space="PSUM") as z_psum,
        ):
            def load(ap_in, shape, tag, dt=f32):
                t = cpool.tile(shape, dt, tag=tag)
                nc.sync.dma_start(out=t[:], in_=ap_in[:, :])
                return t

            idx = load(idx_in, [P, TOT_TILES * 8], "c_idx", i16)
            gcnt = load(gcnt_in, [1, max(NSUBTOT, 1)], "c_gcnt",
                        mybir.dt.int32)
            dstloc = load(dstloc_in, [P, NJOBS], "c_dstloc")
            dstdis = load(dstdis_in, [P, NJOBS], "c_dstdis")
            cnt_regs = [nc.gpsimd.alloc_register(f"cntreg{i}")
                        for i in range(4)]
            cnt_rr = [0]

            def load_cnt(col):
                r = cnt_regs[cnt_rr[0] % 4]
                cnt_rr[0] += 1
                nc.gpsimd.reg_load(r, gcnt[0:1, col:col + 1])
                return r
            iota = load(iota_in, [P, P], "c_iota", bf16)
            dis_sc = load(dis_in, [P, G], "c_dis")
            xT = load(xT_in, [P, G * P], "c_xT", bf16)
            W = [load(w, [P, P], f"c_W{i}", bf16) for i, w in enumerate(w_in)]
            BB = ([load(b, [P, 1], f"c_BB{i}") for i, b in enumerate(bb_in[:4])]
                  + [load(bb_in[4], [P, P], "c_BB4")] if use_bias else None)

            def own_rows(g, z_ps, pi):
                """scale z (PSUM [128, D]) by dis, store bf16 rows, fire AGs."""
                rows = P if g < G - 1 else last_rows
                c = chunk_of_group(g)
                r0 = g * P - int(crow_start[c])
                h = act_pool.tile([P, D], bf16, tag="hrow")
                nc.scalar.mul(h[:], z_ps[:], dis_sc[:, g:g + 1])
                nc.sync.dma_start(out=h_own[c][r0:r0 + rows, :],
                                  in_=h[:rows, :])
                if g == int(gstart[c + 1]) - 1 and not skip_ag:
                    nc.gpsimd.collective_compute(
                        "AllGather", mybir.AluOpType.bypass,
                        replica_groups=RG,
                        ins=[h_own[c].ap().opt()],
                        outs=[tbl[pi % 2][c].ap().opt()])

            for _rep in range(repeat):
                # ---- prologue: table0 rows = dis * (x @ W0)
                for g in range(G):
                    z = z_psum.tile([P, D], f32, space="PSUM")
                    nc.tensor.matmul(out=z[:], lhsT=xT[:, g * P:(g + 1) * P],
                                     rhs=W[0][:], start=True, stop=True)
                    own_rows(g, z, 0)

                passes = [("selu", 1), ("silu", 2), ("silu", 3),
                          ("softplus_neg", 4), ("final", None)]
                passes = passes[:n_passes]
                for pi, (fn, wnext) in enumerate(passes):
                    tblp = tbl[pi % 2]
                    subcol = [0]
                    for b in range(NB):
                        glo = b * GB
                        ghi = min(glo + GB, G)
                        ng = ghi - glo
                        # ---- gathers: one per (batch, chunk)
                        msgs = []
                        for c in range(NCHUNK):
                            tm = int(Tmax[b, c])
                            if not tm:
                                msgs.append(None)
                                continue
                            m = msg_pool.tile([P, TMAXBUF, D], bf16)
                            o = int(tile_off[b, c])
                            t0m = int(plan["cnt_min"][b, c]) // P
                            if t0m < tm:
                                nc.vector.memset(m[:, t0m:tm, :], 0.0)
                            for s in range((tm + SUB - 1) // SUB):
                                t0s = s * SUB
                                t1s = min(t0s + SUB, tm)
                                nt = t1s - t0s
                                if not skip_gather:
                                    nv = load_cnt(subcol[0])
                                    # chunk c owns its own queue subset: a
                                    # pending AG on chunk 1 never blocks
                                    # chunk-0 gathers
                                    if QSPLIT and NQUEUES >= 2 * NCHUNK:
                                        qpc = NQUEUES // NCHUNK
                                        qn = c * qpc + s % qpc
                                    else:
                                        qn = subcol[0] % NQUEUES
                                    nc.gpsimd.dma_gather(
                                        m[:, t0s:t1s, :],
                                        tblp[c][0:NCORES * crows[c], :],
                                        idx[:, (o + t0s) * 8:(o + t1s) * 8],
                                        nt * P, nv, D,
                                        single_packet=nt * P <= 128,
                                        queue_num=qn)
                                subcol[0] += 1
                            msgs.append(m)
                        # ---- indicator matmuls
                        psb = agg_psum.tile([P, GB * D], f32, space="PSUM",
                                            name="aggps", tag="aggps")
                        ps = [psb[:, s * D:(s + 1) * D] for s in range(ng)]
                        seq = jobs_all[b]
                        first = {}
                        last = {}
                        for j, (c, t, s) in enumerate(seq):
                            first.setdefault(s, j)
                            last[s] = j
                        col0 = sum(len(jobs_all[x]) for x in range(b))
                        flipped = wnext is not None
                        for j, (c, t, s) in enumerate(seq):
                            if s >= ng:
                                continue
                            ind = ind_pool.tile([P, P], bf16)
                            if not skip_ind:
                                # indicator with dis[dst] folded in:
                                # ind[e, d] = (d == dst[e]) * dis[dst[e]]
                                nc.vector.tensor_scalar(
                                    out=ind[:], in0=iota[:],
                                    scalar1=dstloc[:, col0 + j:col0 + j + 1],
                                    scalar2=dstdis[:, col0 + j:col0 + j + 1],
                                    op0=mybir.AluOpType.is_equal,
                                    op1=mybir.AluOpType.mult)
                            if not skip_mm or first[s] == j or last[s] == j:
                                if flipped:
                                    # psT[f, d] += msg[e, f]^T @ ind[e, d]
                                    nc.tensor.matmul(out=ps[s][:],
                                                     lhsT=msgs[c][:, t, :],
                                                     rhs=ind[:],
                                                     start=(first[s] == j),
                                                     stop=(last[s] == j))
                                else:
                                    # ps[d, f] += ind[e, d]^T @ msg[e, f]
                                    nc.tensor.matmul(out=ps[s][:], lhsT=ind[:],
                                                     rhs=msgs[c][:, t, :],
                                                     start=(first[s] == j),
                                                     stop=(last[s] == j))
                        # ---- evacuation: batch-wide activations (dis folded
                        # into the indicator), then per group: next-layer
                        # matmul + own-rows (or output DMA)
                        ngD = ng * D
                        pw = psb[:, 0:ngD]
                        if flipped:
                            # psT [f, d]; actw serves directly as next lhsT
                            actw = act_pool.tile([P, GB * D], bf16, tag="actw")
                            aw = actw[:, 0:ngD]
                            if use_bias:
                                lin = tmp_pool.tile([P, GB * D], f32,
                                                    tag="lin")
                                nc.vector.tensor_scalar(
                                    out=lin[:, 0:ngD], in0=pw,
                                    scalar1=BB[pi][:, 0:1], scalar2=None,
                                    op0=mybir.AluOpType.add)
                                srcw = lin[:, 0:ngD]
                            else:
                                srcw = pw
                            if fn == "silu":
                                if sim_safe:
                                    sg = tmp_pool.tile([P, GB * D], f32,
                                                       tag="sg")
                                    xx = tmp_pool.tile([P, GB * D], f32,
                                                       tag="xx")
                                    nc.scalar.activation(sg[:, 0:ngD], srcw,
                                                         AF.Sigmoid)
                                    nc.scalar.mul(xx[:, 0:ngD], srcw, 1.0)
                                    nc.vector.tensor_tensor(
                                        out=aw, in0=sg[:, 0:ngD],
                                        in1=xx[:, 0:ngD],
                                        op=mybir.AluOpType.mult)
                                else:
                                    nc.scalar.activation(aw, srcw, AF.Silu)
                            elif fn == "softplus_neg":
                                e = tmp_pool.tile([P, GB * D], f32,
                                                  tag="sp_e")
                                nc.scalar.activation(e[:, 0:ngD], srcw,
                                                     AF.Exp, scale=-1.0)
                                nc.scalar.activation(aw, e[:, 0:ngD], AF.Ln,
                                                     bias=1.0)
                            else:  # selu
                                r = tmp_pool.tile([P, GB * D], f32,
                                                  tag="selu_r")
                                mm = tmp_pool.tile([P, GB * D], f32,
                                                   tag="selu_m")
                                nc.scalar.activation(r[:, 0:ngD], srcw,
                                                     AF.Relu, scale=SELU_L)
                                nc.scalar.activation(mm[:, 0:ngD], srcw,
                                                     AF.Relu, scale=-1.0)
                                nc.scalar.activation(mm[:, 0:ngD],
                                                     mm[:, 0:ngD],
                                                     AF.Exp, scale=-1.0)
                                nc.vector.tensor_scalar(
                                    out=mm[:, 0:ngD], in0=mm[:, 0:ngD],
                                    scalar1=SELU_L * SELU_A,
                                    scalar2=-SELU_L * SELU_A,
                                    op0=mybir.AluOpType.mult,
                                    op1=mybir.AluOpType.add)
                                nc.vector.tensor_tensor(
                                    out=aw, in0=r[:, 0:ngD],
                                    in1=mm[:, 0:ngD],
                                    op=mybir.AluOpType.add)
                            for s in range(ng):
                                g = glo + s
                                z = z_psum.tile([P, D], f32, space="PSUM")
                                nc.tensor.matmul(
                                    out=z[:], lhsT=actw[:, s * D:(s + 1) * D],
                                    rhs=W[wnext][:], start=True, stop=True)
                                own_rows(g, z, pi + 1)
                        else:
                            # final pass (unflipped): ps [d, f]
                            actw = act_pool.tile([P, GB * D], f32, tag="actf")
                            nc.vector.tensor_copy(actw[:, 0:ngD], pw)
                            if use_bias:
                                for s in range(ng):
                                    nc.vector.tensor_tensor(
                                        out=actw[:, s * D:(s + 1) * D],
                                        in0=actw[:, s * D:(s + 1) * D],
                                        in1=BB[4][:],
                                        op=mybir.AluOpType.add)
                            for s in range(ng):
                                g = glo + s
                                rows = P if g < G - 1 else last_rows
                                nc.sync.dma_start(
                                    out=mu_out[g * P:g * P + rows, :],
                                    in_=actw[:rows, s * D:s * D + OUTC])
                                nc.sync.dma_start(
                                    out=lv_out[g * P:g * P + rows, :],
                                    in_=actw[:rows, s * D + OUTC:(s + 1) * D])
    nc.finalize()
    return nc


# ------------------------------------------------------------------- driver
def _make_in_maps(x, plan, per_core, Ws, biases=None):
    NPC, G, last_rows, *_ = _dims()
    iota = np.tile(np.arange(P, dtype=np.float32), (P, 1))
    bfdt = mybir.dt.np(bf16)
    in_maps = []
    for k in range(NCORES):
        pc = per_core[k]
        idx, dstloc, dstdis, gcnt = _core_inputs(plan, pc)
        dis_k = pc["dis"]
        xT = np.zeros((P, G * P), np.float32)
        xT[:, :NPC] = x[k * NPC:(k + 1) * NPC].T
        m = dict(idx=idx, gcnt=gcnt, dstloc=dstloc.astype(np.float32),
                 dstdis=dstdis.astype(np.float32),
                 iota=iota.astype(bfdt), dis_sc=dis_k,
                 xT=xT.astype(bfdt))
        for i, w in enumerate(Ws):
            m[f"W{i}"] = np.asarray(w, np.float32).astype(bfdt)
        if biases is not None:
            for i, b in enumerate(biases):
                bv = np.asarray(b, dtype=np.float32)
                if i < 4:
                    m[f"BB{i}"] = bv.reshape(P, 1)
                else:
                    m[f"BB{i}"] = np.tile(bv[None, :], (P, 1))
        in_maps.append(m)
    return in_maps


def kernel(x, edge_index, W0, b0, W1, b1, W2, b2, W3, b3, Wmu, bmu, Wlv, blv):
    x = np.asarray(x, dtype=np.float32)
    edge_index = np.asarray(edge_index)
    assert x.shape == (N, D) and edge_index.shape == (2, E)

    plan, per_core, _dis = _preprocess(edge_index)
    use_bias = any(np.any(np.asarray(b)) for b in (b0, b1, b2, b3, bmu, blv))

    key = (plan["Tmax"].tobytes(),
           tuple(tuple(j) for j in plan["jobs"][0]), use_bias)
    if key not in _CACHE:
        _CACHE[key] = _build(plan, use_bias)
    nc = _CACHE[key]

    Wmulv = np.concatenate([-np.asarray(Wmu), -np.asarray(Wlv)],
                           axis=1).astype(np.float32)
    Ws = [np.asarray(w, dtype=np.float32) for w in (W0, W1, W2, W3)] + [Wmulv]
    biases = None
    if use_bias:
        bmulv = np.concatenate([np.asarray(bmu), np.asarray(blv)])
        biases = (b0, b1, b2, b3, bmulv)
    in_maps = _make_in_maps(x, plan, per_core, Ws, biases)

    res = run_bass_kernel_spmd(nc, in_maps, core_ids=list(range(NCORES)))
    mu = np.concatenate([res.results[k]["mu_out"] for k in range(NCORES)],
                        axis=0)
    lv = np.concatenate([res.results[k]["lv_out"] for k in range(NCORES)],
                        axis=0)
    return (mu, lv)

